# revision 1
# baseline (speedup 1.0000x reference)
"""Bass/Tile TRN2 kernel for nn_GCN_17008070492360.

N,K,F = 100000,16,128; H1,H2,C = 64,32,10. Data-parallel over nodes on
8 NeuronCores (padded to 12544 nodes/core); W1/W2/Wc replicated (bf16).

Per-core pipeline (all engines overlapped, Tile-scheduled):
  DMA   : contiguous 512KB row-tile loads ([128 rows, 128] fp32)
  DVE/ACT: per-row sum-of-squares (STT-accum / Square-accum split),
           sqrt(ss+eps), 1/x, fused normalize+cast->bf16
  PE    : transpose-mode matmuls (bf16) -> psum; W1 projection with the
          x1 term added via an identity-lhsT matmul over a stride-0
          replicated AP; stage-2 W2 matmul fused with the k-sum as 16
          PSUM-accumulating matmuls; Wc matmul
  ACT   : relu (psum->sbuf, applied 128x512 at a time)
Output is written transposed ([2C, NP/2] slabs) and un-permuted on host.
"""

import sys
from contextlib import ExitStack

for _p in ("/opt/trn_rl_repo",):
    if _p not in sys.path:
        sys.path.insert(0, _p)

import numpy as np
import ml_dtypes

import concourse.tile as tile
from concourse import bacc, mybir

dt = mybir.dt
AF = mybir.ActivationFunctionType
ALU = mybir.AluOpType

F = 128
K = 16
H1 = 64
H2 = 32
C = 10
NCORES = 8
BB_SBS = 14
NBB = 14
NP = 64 * BB_SBS * NBB  # 12544 nodes per core (100352 padded total)


def _build(copyback_split=3, ss_act=2):
    SB = 64
    XT = NP // 128
    R = NP * K

    nc = bacc.Bacc("TRN2", target_bir_lowering=False)

    x_d = nc.dram_tensor("x", [NP, F], dt.float32, kind="ExternalInput")
    nb_d = nc.dram_tensor("nb", [R, F], dt.float32, kind="ExternalInput")
    w1t_d = nc.dram_tensor("w1t", [F, H1], dt.bfloat16, kind="ExternalInput")
    w2t_d = nc.dram_tensor("w2t2", [2 * H1, H2], dt.bfloat16, kind="ExternalInput")
    wct_d = nc.dram_tensor("wct2", [2 * H2, C], dt.bfloat16, kind="ExternalInput")
    i128_d = nc.dram_tensor("i128", [128, 128], dt.bfloat16, kind="ExternalInput")
    i64_d = nc.dram_tensor("i64", [64, 64], dt.bfloat16, kind="ExternalInput")
    outT_d = nc.dram_tensor(
        "outT", [2 * C, NP // 2], dt.float32, kind="ExternalOutput"
    )

    with tile.TileContext(nc) as tc, ExitStack() as ctx:
        cpool = ctx.enter_context(tc.tile_pool(name="consts", bufs=1))
        ppool = ctx.enter_context(tc.tile_pool(name="persist", bufs=1))
        inpool = ctx.enter_context(tc.tile_pool(name="in", bufs=4))
        bfpool = ctx.enter_context(tc.tile_pool(name="bf", bufs=4))
        tpool = ctx.enter_context(tc.tile_pool(name="tp", bufs=4))
        spool = ctx.enter_context(tc.tile_pool(name="small", bufs=8))
        rpool = ctx.enter_context(tc.tile_pool(name="relu", bufs=4))
        s23pool = ctx.enter_context(tc.tile_pool(name="s23", bufs=2))
        pppool = ctx.enter_context(tc.tile_pool(name="psumT", bufs=3, space="PSUM"))
        papool = ctx.enter_context(tc.tile_pool(name="psumAB", bufs=3, space="PSUM"))
        pzpool = ctx.enter_context(tc.tile_pool(name="psumZ", bufs=2, space="PSUM"))

        w1t = cpool.tile([F, H1], dt.bfloat16)
        nc.sync.dma_start(w1t[:], w1t_d[:, :])
        w2t = cpool.tile([2 * H1, H2], dt.bfloat16)
        nc.sync.dma_start(w2t[:], w2t_d[:, :])
        wct = cpool.tile([2 * H2, C], dt.bfloat16)
        nc.sync.dma_start(wct[:], wct_d[:, :])
        i128 = cpool.tile([128, 128], dt.bfloat16)
        nc.sync.dma_start(i128[:], i128_d[:, :])
        i64 = cpool.tile([64, 64], dt.bfloat16)
        nc.sync.dma_start(i64[:], i64_d[:, :])
        eps = cpool.tile([128, 1], dt.float32)
        nc.gpsimd.memset(eps[:], 1e-24)

        x1T = ppool.tile([H1, NP], dt.bfloat16, tag="x1T")

        # ---- phase X: x1T = (l2norm(x) @ W1.T).T, bf16 [H1, NP]
        XG = 4 if XT % 4 == 0 else (2 if XT % 2 == 0 else 1)
        for g in range(XT // XG):
            natf = inpool.tile([128, XG * F], dt.float32, tag="xnat")
            src = x_d[g * XG * 128 : (g + 1) * XG * 128, :].rearrange(
                "(t p) f -> p t f", p=128
            )
            nc.sync.dma_start(natf[:].rearrange("p (t f) -> p t f", t=XG), src)
            ss = spool.tile([128, XG], dt.float32, tag="xss")
            junk = bfpool.tile([128, XG * F], dt.bfloat16, tag="xjunk")
            for t in range(XG):
                nc.vector.scalar_tensor_tensor(
                    out=junk[:, t * F : (t + 1) * F],
                    in0=natf[:, t * F : (t + 1) * F],
                    scalar=1.0,
                    in1=natf[:, t * F : (t + 1) * F],
                    op0=ALU.mult,
                    op1=ALU.mult,
                    accum_out=ss[:, t : t + 1],
                )
            nrm = spool.tile([128, XG], dt.float32, tag="xnrm")
            nc.scalar.activation(nrm[:], ss[:], AF.Sqrt, bias=eps[:])
            r = spool.tile([128, XG], dt.float32, tag="xr")
            nc.vector.reciprocal(r[:], nrm[:])
            natn = bfpool.tile([128, XG * F], dt.bfloat16, tag="xnatn")
            for t in range(XG):
                nc.vector.tensor_scalar_mul(
                    natn[:, t * F : (t + 1) * F],
                    natf[:, t * F : (t + 1) * F],
                    r[:, t : t + 1],
                )
            psT = pppool.tile([128, XG * F], dt.bfloat16, tag="psT")
            for t in range(XG):
                nc.tensor.transpose(
                    psT[:, t * F : (t + 1) * F],
                    natn[:, t * F : (t + 1) * F],
                    i128[:],
                )
            xT = tpool.tile([128, XG * F], dt.bfloat16, tag="xT")
            nc.scalar.copy(xT[:], psT[:])
            psX = papool.tile([H1, XG * 128], dt.float32, tag="psAB")
            for t in range(XG):
                nc.tensor.matmul(
                    psX[:, t * 128 : (t + 1) * 128],
                    lhsT=w1t[:],
                    rhs=xT[:, t * 128 : (t + 1) * 128],
                    start=True,
                    stop=True,
                )
            nc.scalar.copy(x1T[:, g * XG * 128 : (g + 1) * XG * 128], psX[:])

        # ---- phase NB (paired 1MB loads)
        assert BB_SBS % 2 == 0
        for bb in range(NBB):
            W = 32 * BB_SBS
            psZ = pzpool.tile([64, W], dt.float32, tag="psZY")
            for sbp in range(BB_SBS // 2):
                sb0 = bb * BB_SBS + 2 * sbp
                rows0 = sb0 * 1024
                natf = inpool.tile([128, 16 * F], dt.float32, tag="nat")
                src = nb_d[rows0 : rows0 + 2048, :].rearrange(
                    "(t p) f -> p t f", p=128
                )
                nc.sync.dma_start(
                    natf[:].rearrange("p (t f) -> p t f", t=16), src
                )

                ss = spool.tile([128, 16], dt.float32, tag="ss")
                junk = bfpool.tile([128, 16 * F], dt.bfloat16, tag="junk")
                for t in range(16):
                    if t % 8 < ss_act:
                        nc.scalar.activation(
                            junk[:, t * F : (t + 1) * F],
                            natf[:, t * F : (t + 1) * F],
                            AF.Square,
                            accum_out=ss[:, t : t + 1],
                        )
                    else:
                        nc.vector.scalar_tensor_tensor(
                            out=junk[:, t * F : (t + 1) * F],
                            in0=natf[:, t * F : (t + 1) * F],
                            scalar=1.0,
                            in1=natf[:, t * F : (t + 1) * F],
                            op0=ALU.mult,
                            op1=ALU.mult,
                            accum_out=ss[:, t : t + 1],
                        )
                nrm = spool.tile([128, 16], dt.float32, tag="nrm")
                nc.scalar.activation(nrm[:], ss[:], AF.Sqrt, bias=eps[:])
                r = spool.tile([128, 16], dt.float32, tag="r")
                nc.vector.reciprocal(r[:], nrm[:])

                for half in range(2):
                    sbi = 2 * sbp + half
                    sb = bb * BB_SBS + sbi
                    o = half * 8
                    natn = bfpool.tile([128, 8 * F], dt.bfloat16, tag="natn")
                    for t in range(8):
                        nc.vector.tensor_scalar_mul(
                            natn[:, t * F : (t + 1) * F],
                            natf[:, (o + t) * F : (o + t + 1) * F],
                            r[:, o + t : o + t + 1],
                        )

                    psT = pppool.tile([128, 8 * F], dt.bfloat16, tag="psT")
                    for t in range(8):
                        nc.tensor.transpose(
                            psT[:, t * F : (t + 1) * F],
                            natn[:, t * F : (t + 1) * F],
                            i128[:],
                        )
                    nbT = tpool.tile([128, 8 * F], dt.bfloat16, tag="nbT")
                    cs = copyback_split * F
                    if copyback_split > 0:
                        nc.vector.tensor_copy(nbT[:, 0:cs], psT[:, 0:cs])
                    if copyback_split < 8:
                        nc.scalar.copy(nbT[:, cs:], psT[:, cs:])

                    psAB = papool.tile([128, 512], dt.float32, tag="psAB")
                    for h in range(2):
                        node0 = sb * SB + 32 * h
                        x1rep = (
                            x1T[:, node0 : node0 + 32]
                            .unsqueeze(2)
                            .broadcast_to([H1, 32, K])
                        )
                        nc.tensor.matmul(
                            psAB[64 * h : 64 * h + 64, :],
                            lhsT=w1t[:],
                            rhs=nbT[:, 512 * h : 512 * h + 512],
                            start=True,
                            stop=False,
                        )
                        nc.tensor.matmul(
                            psAB[64 * h : 64 * h + 64, :],
                            lhsT=i64[:],
                            rhs=x1rep,
                            start=False,
                            stop=True,
                        )
                    relu = rpool.tile([128, 512], dt.bfloat16, tag="relu")
                    nc.scalar.activation(relu[:], psAB[:], AF.Relu)
                    for h in range(2):
                        relu_r = relu[64 * h : 64 * h + 64, :].rearrange(
                            "p (n k) -> p k n", k=K
                        )
                        for k in range(K):
                            nc.tensor.matmul(
                                psZ[
                                    32 * h : 32 * h + 32,
                                    32 * sbi : 32 * sbi + 32,
                                ],
                                lhsT=w2t[64 * h : 64 * h + 64, :],
                                rhs=relu_r[:, k, :],
                                start=(k == 0),
                                stop=(k == K - 1),
                                skip_group_check=True,
                            )

            relu2 = s23pool.tile([64, W], dt.bfloat16, tag="relu2")
            nc.scalar.activation(relu2[:], psZ[:], AF.Relu)
            psY = pzpool.tile([64, W], dt.float32, tag="psZY")
            for h in range(2):
                nc.tensor.matmul(
                    psY[32 * h : 32 * h + C, :],
                    lhsT=wct[32 * h : 32 * h + 32, :],
                    rhs=relu2[32 * h : 32 * h + 32, :],
                    start=True,
                    stop=True,
                    tile_position=(32 * h, 32 * h),
                )
            yT = s23pool.tile([64, W], dt.float32, tag="yT")
            nc.scalar.copy(yT[0:C, :], psY[0:C, :])
            nc.scalar.copy(yT[32 : 32 + C, :], psY[32 : 32 + C, :])
            nc.sync.dma_start(outT_d[0:C, bb * W : (bb + 1) * W], yT[0:C, :])
            nc.sync.dma_start(
                outT_d[C : 2 * C, bb * W : (bb + 1) * W], yT[32 : 32 + C, :]
            )

    nc.compile()
    return nc


_NC = None


def _get_nc():
    global _NC
    if _NC is None:
        _NC = _build()
    return _NC


def kernel(x, neighbor, W1, W2, Wc):
    from concourse.bass_utils import run_bass_kernel_spmd

    nc = _get_nc()
    x = np.ascontiguousarray(np.asarray(x, np.float32))
    neighbor = np.ascontiguousarray(np.asarray(neighbor, np.float32))
    N = x.shape[0]
    Npad = NP * NCORES
    xp = np.zeros((Npad, F), np.float32)
    xp[:N] = x
    nbp = np.zeros((Npad, K, F), np.float32)
    nbp[:N] = neighbor

    w1t = np.ascontiguousarray(np.asarray(W1, np.float32).T).astype(
        ml_dtypes.bfloat16
    )
    w2t1 = np.ascontiguousarray(np.asarray(W2, np.float32).T).astype(
        ml_dtypes.bfloat16
    )
    wct1 = np.ascontiguousarray(np.asarray(Wc, np.float32).T).astype(
        ml_dtypes.bfloat16
    )
    w2t = np.vstack([w2t1, w2t1])
    wct = np.vstack([wct1, wct1])
    i128 = np.eye(128, dtype=ml_dtypes.bfloat16)
    i64 = np.eye(64, dtype=ml_dtypes.bfloat16)

    in_maps = []
    for c in range(NCORES):
        in_maps.append(
            {
                "x": xp[c * NP : (c + 1) * NP],
                "nb": nbp[c * NP : (c + 1) * NP].reshape(NP * K, F),
                "w1t": w1t,
                "w2t2": w2t,
                "wct2": wct,
                "i128": i128,
                "i64": i64,
            }
        )

    res = run_bass_kernel_spmd(nc, in_maps, core_ids=list(range(NCORES)))

    # un-permute: node = 64*sbg + 32*half + i -> outT[10*half + c, 32*sbg + i]
    out = np.empty((Npad, C), np.float32)
    n = np.arange(NP)
    col = 32 * (n // 64) + (n % 32)
    row0 = C * ((n % 64) // 32)
    for c in range(NCORES):
        oT = res.results[c]["outT"]
        for cls in range(C):
            out[c * NP + n, cls] = oT[row0 + cls, col]
    return out[:N]



# revision 37
# speedup vs baseline: 1.2254x; 1.2254x over previous
"""Bass/Tile TRN2 kernel for nn_GCN_17008070492360.

N,K,F = 100000,16,128; H1,H2,C = 64,32,10. Data-parallel over nodes on
8 NeuronCores (12544 nodes/core); weights replicated (bf16).

Host prep: x/neighbor cast to bf16 and uploaded row-PERMUTED + row-paired
([R/2, 256] lines of 512B) so that flat contiguous DMA loads land rows in
the exact column order the device pipeline needs after PE transpose.

Per-core pipeline per 2048-row unit (engines balanced, Tile-scheduled):
  SP/DMA : one contiguous 512KB bf16 load
  DVE    : sum-of-squares STT slices (split with Pool/ACT), reciprocal,
           per-row normalize (tensor_scalar, bf16 4x), psum copyback share
  ACT    : sqrt(ss+eps), ReLU on [128,1024] psum, copyback share, bb tails
  PE     : 16 transposes, W1 matmuls, W2 k-sum accumulating matmuls, Wc
  Pool   : x1-add STTs on psum (x1 replicated+32-shifted buffer), ss share
Output written transposed ([2C, NP/2] slabs) and un-permuted on host.
"""

import sys
from contextlib import ExitStack

for _p in ("/opt/trn_rl_repo",):
    if _p not in sys.path:
        sys.path.insert(0, _p)

import numpy as np
import ml_dtypes

import concourse.tile as tile
from concourse import bacc, mybir

dt = mybir.dt
AF = mybir.ActivationFunctionType
ALU = mybir.AluOpType

F = 128
K = 16
H1 = 64
H2 = 32
C = 10
NCORES = 8
UNITS = 98          # nb units of 2048 rows (16 slices of 128)
BB_UNITS = 7        # units per bb -> 14 sb per bb
NBB = UNITS // BB_UNITS  # 14
NP = UNITS * 128    # 12544 nodes per core
XUNITS = [2048] * 6 + [256]  # x rows per x-unit (sum = 12544)

# tunable work splits (counts out of 16 ss slices)
SS_POOL = 0
SS_ACT = 2
# leftover PE-transpose copyback: 0 -> DVE, >=1 -> ACT
CB_ACT_HALVES = 1
# slices (of 16) transposed via one DMA-xbar instruction; rest via PE
TR_DMA_SLICES = 0
# x1-add: how many of the 2 sb-blocks per unit go to Pool STT (rest PE matmul)
X1_POOL_SB = 0
# norm slices done on Pool via tensor_tensor broadcast (0 if Pool lacks tt)
NORM_POOL = 10


def _build():
    nc = bacc.Bacc("TRN2", target_bir_lowering=False)

    x_d = nc.dram_tensor("x2", [NP // 2, 2 * F], dt.bfloat16, kind="ExternalInput")
    nb_d = nc.dram_tensor(
        "nb2", [NP * K // 2, 2 * F], dt.bfloat16, kind="ExternalInput"
    )
    w1t_d = nc.dram_tensor("w1t", [F, H1], dt.bfloat16, kind="ExternalInput")
    w2t_d = nc.dram_tensor("w2t2", [2 * H1, 2 * H2], dt.bfloat16, kind="ExternalInput")
    wct_d = nc.dram_tensor("wct2", [2 * H2, 32 + C], dt.bfloat16, kind="ExternalInput")
    i128_d = nc.dram_tensor("i128", [128, 128], dt.bfloat16, kind="ExternalInput")
    i64_d = nc.dram_tensor("i64", [64, 64], dt.bfloat16, kind="ExternalInput")
    outT_d = nc.dram_tensor(
        "outT", [2 * C, NP // 2], dt.float32, kind="ExternalOutput"
    )

    with tile.TileContext(nc) as tc, ExitStack() as ctx:
        cpool = ctx.enter_context(tc.tile_pool(name="consts", bufs=1))
        ppool = ctx.enter_context(tc.tile_pool(name="persist", bufs=1))
        inpool = ctx.enter_context(tc.tile_pool(name="in", bufs=8))
        jpool = ctx.enter_context(tc.tile_pool(name="junk", bufs=6))
        npool = ctx.enter_context(tc.tile_pool(name="natn", bufs=8))
        tpool = ctx.enter_context(tc.tile_pool(name="nbT", bufs=6))
        spool = ctx.enter_context(tc.tile_pool(name="small", bufs=16))
        rpool = ctx.enter_context(tc.tile_pool(name="relu", bufs=4))
        s23pool = ctx.enter_context(tc.tile_pool(name="s23", bufs=2))
        pppool = ctx.enter_context(tc.tile_pool(name="psumT", bufs=2, space="PSUM"))
        papool = ctx.enter_context(tc.tile_pool(name="psumAB", bufs=2, space="PSUM"))
        pzpool = ctx.enter_context(tc.tile_pool(name="psumZ", bufs=2, space="PSUM"))

        w1t = cpool.tile([F, H1], dt.bfloat16)
        nc.sync.dma_start(w1t[:], w1t_d[:, :])
        w2t = cpool.tile([2 * H1, 2 * H2], dt.bfloat16)
        nc.sync.dma_start(w2t[:], w2t_d[:, :])
        wct = cpool.tile([2 * H2, 32 + C], dt.bfloat16)
        nc.sync.dma_start(wct[:], wct_d[:, :])
        i128 = cpool.tile([128, 128], dt.bfloat16)
        nc.sync.dma_start(i128[:], i128_d[:, :])
        i64 = cpool.tile([64, 64], dt.bfloat16)
        nc.sync.dma_start(i64[:], i64_d[:, :])
        eps = cpool.tile([128, 1], dt.float32)
        nc.gpsimd.memset(eps[:], 1e-24)

        # x1 duplicated in both partition halves; half2 shifted by +32 nodes
        x1T2b = ppool.tile([128, NP], dt.bfloat16, tag="x1T2b")

        def stage_a(src_d, row0, rows):
            """Load rows, compute ss -> r = 1/sqrt(ss+eps). Returns state."""
            nsl = rows // 128
            natf_t = inpool.tile([128, 2048], dt.bfloat16, tag="natf")
            natf = natf_t[:, 0:rows]
            src = src_d[row0 // 2 : (row0 + rows) // 2, :].rearrange(
                "(t p) f -> p t f", p=128
            )
            nc.sync.dma_start(
                natf.rearrange("p (t f) -> p t f", t=rows // 256), src
            )
            ss = spool.tile([128, 16], dt.float32, tag="ss")
            junk_t = jpool.tile([128, 2048], dt.bfloat16, tag="junk")
            junk = junk_t[:, 0:rows]
            for c in range(nsl):
                sl = slice(c * 128, (c + 1) * 128)
                cc = c % 8
                half0 = (c % 16) < 8
                ph = (SS_POOL + 1) // 2 if half0 else SS_POOL // 2
                pa = SS_ACT // 2 if half0 else (SS_ACT + 1) // 2
                if cc < ph:
                    nc.gpsimd.scalar_tensor_tensor(
                        out=junk[:, sl], in0=natf[:, sl], scalar=1.0,
                        in1=natf[:, sl], op0=ALU.mult, op1=ALU.mult,
                        accum_out=ss[:, c : c + 1],
                    )
                elif cc < ph + pa:
                    nc.scalar.activation(
                        junk[:, sl], natf[:, sl], AF.Square,
                        accum_out=ss[:, c : c + 1],
                    )
                else:
                    nc.vector.scalar_tensor_tensor(
                        out=junk[:, sl], in0=natf[:, sl], scalar=1.0,
                        in1=natf[:, sl], op0=ALU.mult, op1=ALU.mult,
                        accum_out=ss[:, c : c + 1],
                    )
            nrm = spool.tile([128, 16], dt.float32, tag="nrm")
            nc.scalar.activation(nrm[:, 0:nsl], ss[:, 0:nsl], AF.Sqrt, bias=eps[:])
            r = spool.tile([128, 16], dt.float32, tag="r")
            nc.vector.reciprocal(r[:, 0:nsl], nrm[:, 0:nsl])
            return {"natf": natf, "r": r, "rows": rows}

        def stage_n(st):
            """Normalize natf by r -> natn (DVE)."""
            natf, r, rows = st["natf"], st["r"], st["rows"]
            nsl = rows // 128
            natn_t = npool.tile([128, 2048], dt.bfloat16, tag="natn")
            natn = natn_t[:, 0:rows]
            for c in range(nsl):
                sl = slice(c * 128, (c + 1) * 128)
                if (c % 16) >= 16 - NORM_POOL:
                    rb = r[:, c : c + 1].broadcast_to([128, 128])
                    nc.gpsimd.tensor_tensor(
                        out=natn[:, sl], in0=natf[:, sl], in1=rb, op=ALU.mult
                    )
                else:
                    nc.vector.tensor_scalar_mul(
                        natn[:, sl], natf[:, sl], r[:, c : c + 1]
                    )
            st["natn"] = natn

        def stage_t(st):
            """Transpose natn -> nbT: DMA-xbar slices (ACT queue) + PE rest."""
            natn, rows = st["natn"], st["rows"]
            nsl = rows // 128
            nbT_t = tpool.tile([128, 2048], dt.bfloat16, tag="nbT")
            nbT = nbT_t[:, 0:rows]
            td = nsl if nsl <= TR_DMA_SLICES else TR_DMA_SLICES
            if td > 0:
                w = td * 128
                nc.scalar.dma_start_transpose(
                    nbT[:, 0:w].rearrange("p (t f) -> p t f", t=td),
                    natn[:, 0:w],
                )
            nleft = nsl - td
            for g0 in range(td, nsl, 8):
                g1 = min(g0 + 8, nsl)
                w = (g1 - g0) * 128
                psT = pppool.tile([128, 1024], dt.bfloat16, tag="psT")
                for c in range(g1 - g0):
                    sl = slice((g0 + c) * 128, (g0 + c + 1) * 128)
                    psl = slice(c * 128, (c + 1) * 128)
                    nc.tensor.transpose(psT[:, psl], natn[:, sl], i128[:])
                osl = slice(g0 * 128, g1 * 128)
                if nleft > 8 and g0 == td and CB_ACT_HALVES >= 1:
                    nc.scalar.copy(nbT[:, osl], psT[:, 0:w])
                else:
                    nc.vector.tensor_copy(nbT[:, osl], psT[:, 0:w])
            st["nbT"] = nbT
            return nbT

        # ---- phase X: x1T2b = W1-projection of normalized x, duplicated in
        # both partition halves (half2 shifted by +32 nodes at copy time)
        def x_project(st):
            xnT, col0, xrows = st["nbT"], st["col"], st["rows"]
            for c0 in range(0, xrows, 512):
                w = min(512, xrows - c0)
                psX_t = papool.tile([128, 1024], dt.float32, tag="psAB")
                psX = psX_t[:, 0:w]
                for h in range(2):
                    nc.tensor.matmul(
                        psX[64 * h : 64 * h + 64, :],
                        lhsT=w1t[:],
                        rhs=xnT[:, c0 : c0 + w],
                        start=True,
                        stop=True,
                    )
                col = col0 + c0
                nc.scalar.copy(x1T2b[0:64, col : col + w], psX[0:64, :])
                lo = max(col - 32, 0)
                skip = lo - (col - 32)
                if w - skip > 0:
                    nc.vector.tensor_copy(
                        x1T2b[64:128, lo : col + w - 32], psX[64:128, skip:w]
                    )

        jobs = []
        xoff = 0
        for xrows in XUNITS:
            jobs.append(("x", xoff, xrows))
            xoff += xrows
        for u in range(UNITS):
            jobs.append(("nb", u, 2048))

        psZ_by_bb = {}
        states = {}

        def stage_c1(u, st):
            """W1 + x1-add matmuls into psAB, then relu halves (ACT)."""
            nbT = st["nbT"]
            node0 = u * 128
            psAB = papool.tile([128, 1024], dt.float32, tag="psAB")
            st["psAB"] = psAB
            relu = rpool.tile([128, 1024], dt.bfloat16, tag="relu")
            st["relu"] = relu
            for sb in range(2):
                n0 = node0 + sb * 64
                pool_x1 = sb < X1_POOL_SB
                for h in range(2):
                    nc.tensor.matmul(
                        psAB[64 * h : 64 * h + 64, sb * 512 : sb * 512 + 512],
                        lhsT=w1t[:],
                        rhs=nbT[:, sb * 1024 + h * 512 : sb * 1024 + h * 512 + 512],
                        start=True,
                        stop=pool_x1,
                        skip_group_check=True,
                    )
                if not pool_x1:
                    x1rep = (
                        x1T2b[:, n0 : n0 + 32]
                        .unsqueeze(2)
                        .broadcast_to([128, 32, K])
                    )
                    nc.tensor.matmul(
                        psAB[:, sb * 512 : sb * 512 + 512],
                        lhsT=i128[:],
                        rhs=x1rep,
                        start=False,
                        stop=True,
                        skip_group_check=True,
                    )
                if pool_x1:
                    x1b = (
                        x1T2b[:, n0 : n0 + 32]
                        .unsqueeze(2)
                        .broadcast_to([128, 32, K])
                    )
                    nc.gpsimd.scalar_tensor_tensor(
                        out=psAB[:, sb * 512 : sb * 512 + 512].rearrange(
                            "p (n k) -> p n k", k=K
                        ),
                        in0=psAB[:, sb * 512 : sb * 512 + 512].rearrange(
                            "p (n k) -> p n k", k=K
                        ),
                        scalar=1.0,
                        in1=x1b,
                        op0=ALU.mult,
                        op1=ALU.add,
                    )


        def stage_c2(u, st):
            """W2 k-sum accumulating matmuls into psZ; bb tail at bb end."""
            relu = st["relu"]
            nc.scalar.activation(relu[:], st["psAB"][:], AF.Relu)
            uu = u % BB_UNITS
            bb = u // BB_UNITS
            for sb in range(2):
                sbi = 2 * uu + sb
                relu_r = relu[:, sb * 512 : sb * 512 + 512].rearrange(
                    "p (n k) -> p k n", k=K
                )
                for k in range(K):
                    nc.tensor.matmul(
                        psZ_by_bb[bb][0:64, 32 * sbi : 32 * sbi + 32],
                        lhsT=w2t[:],
                        rhs=relu_r[:, k, :],
                        start=(k == 0),
                        stop=(k == K - 1),
                        skip_group_check=True,
                    )
            if uu == BB_UNITS - 1:
                W = 64 * BB_UNITS
                psZ = psZ_by_bb.pop(bb)
                relu2 = s23pool.tile([64, W], dt.bfloat16, tag="relu2")
                nc.scalar.activation(relu2[:], psZ[:], AF.Relu)
                psY = pzpool.tile([64, W], dt.float32, tag="psZY", name="psY")
                nc.tensor.matmul(
                    psY[0 : 32 + C, :],
                    lhsT=wct[:],
                    rhs=relu2[:],
                    start=True,
                    stop=True,
                )
                yT = s23pool.tile([64, W], dt.float32, tag="yT")
                nc.scalar.copy(yT[0:C, :], psY[0:C, :])
                nc.scalar.copy(yT[32 : 32 + C, :], psY[32 : 32 + C, :])
                nc.sync.dma_start(
                    outT_d[0:C, bb * W : (bb + 1) * W], yT[0:C, :]
                )
                nc.sync.dma_start(
                    outT_d[C : 2 * C, bb * W : (bb + 1) * W], yT[32 : 32 + C, :]
                )


        def do_a(j):
            kind, key, rows = jobs[j]
            if kind == "x":
                st = stage_a(x_d, key, rows)
                st["col"] = key
            else:
                st = stage_a(nb_d, key * 2048, rows)
            states[j] = st

        def do_c(j):
            kind, key, rows = jobs[j]
            st = states.pop(j)
            if kind == "x":
                x_project(st)
            else:
                if key % BB_UNITS == 0:
                    psZ_by_bb[key // BB_UNITS] = pzpool.tile(
                        [64, 64 * BB_UNITS], dt.float32, tag="psZY", name="psZ"
                    )
                stage_c1(key, st)
                stage_c2(key, st)

        NJ = len(jobs)
        for it in range(NJ + 2):
            if it < NJ:
                do_a(it)
            if 0 <= it - 1 < NJ:
                stage_n(states[it - 1])
                stage_t(states[it - 1])
            if 0 <= it - 2:
                do_c(it - 2)

    nc.compile()
    return nc


_NC = None


def _get_nc():
    global _NC
    if _NC is None:
        _NC = _build()
    return _NC


def _nb_row_perm():
    """upload row q (core-local, q in [0, NP*K)) takes original flat row
    perm[q] of nb.reshape(NP*K, F).  Derived from: device col j of unit u
    = slice c=j//128, partition p=j%128; upload row r-in-slab =
    2*((c//2)*128 + p) + c%2; expected (node,k) at col j."""
    j = np.arange(2048)
    node_off = (j // 1024) * 64 + ((j % 1024) // 512) * 32 + ((j % 512) // 16)
    kk = j % 16
    c = j // 128
    p = j % 128
    r = 2 * ((c // 2) * 128 + p) + (c % 2)
    # expected original row (within slab) at upload position r
    exp = np.empty(2048, np.int64)
    exp[r] = node_off * 16 + kk  # original flat row within the 2048-row slab
    u = np.arange(UNITS)[:, None]
    perm = (u * 2048 + exp[None, :]).reshape(-1)
    return perm


def _x_row_perm():
    """upload row q takes original x row perm[q] (core-local)."""
    perm = np.empty(NP, np.int64)
    xoff = 0
    for xrows in XUNITS:
        j = np.arange(xrows)
        c = j // 128
        p = j % 128
        r = 2 * ((c // 2) * 128 + p) + (c % 2)
        exp = np.empty(xrows, np.int64)
        exp[r] = j  # node at col j is xoff + j
        perm[xoff : xoff + xrows] = xoff + exp
        xoff += xrows
    return perm


_NB_PERM = None
_X_PERM = None


def kernel(x, neighbor, W1, W2, Wc):
    from concourse.bass_utils import run_bass_kernel_spmd

    global _NB_PERM, _X_PERM
    nc = _get_nc()
    if _NB_PERM is None:
        _NB_PERM = _nb_row_perm()
        _X_PERM = _x_row_perm()

    x = np.asarray(x, np.float32)
    neighbor = np.asarray(neighbor, np.float32)
    N = x.shape[0]
    Npad = NP * NCORES
    xp = np.zeros((Npad, F), np.float32)
    xp[:N] = x
    nbp = np.zeros((Npad * K, F), np.float32)
    nbp[: N * K] = neighbor.reshape(N * K, F)

    xp = xp.astype(ml_dtypes.bfloat16)
    nbp = nbp.astype(ml_dtypes.bfloat16)

    w1t = np.ascontiguousarray(np.asarray(W1, np.float32).T).astype(
        ml_dtypes.bfloat16
    )
    H1v, H2v, Cv = 64, 32, 10
    w2t1 = np.ascontiguousarray(np.asarray(W2, np.float32).T).astype(
        ml_dtypes.bfloat16
    )
    wct1 = np.ascontiguousarray(np.asarray(Wc, np.float32).T).astype(
        ml_dtypes.bfloat16
    )
    w2t = np.zeros((2 * H1v, 2 * H2v), ml_dtypes.bfloat16)
    w2t[:H1v, :H2v] = w2t1
    w2t[H1v:, H2v:] = w2t1
    wct = np.zeros((2 * H2v, 32 + Cv), ml_dtypes.bfloat16)
    wct[:H2v, :Cv] = wct1
    wct[H2v:, 32 : 32 + Cv] = wct1
    i128 = np.eye(128, dtype=ml_dtypes.bfloat16)
    i64 = np.eye(64, dtype=ml_dtypes.bfloat16)

    in_maps = []
    for cid in range(NCORES):
        nb_local = nbp[cid * NP * K : (cid + 1) * NP * K]
        x_local = xp[cid * NP : (cid + 1) * NP]
        nb_up = np.ascontiguousarray(nb_local[_NB_PERM]).reshape(NP * K // 2, 2 * F)
        x_up = np.ascontiguousarray(x_local[_X_PERM]).reshape(NP // 2, 2 * F)
        in_maps.append(
            {
                "x2": x_up,
                "nb2": nb_up,
                "w1t": w1t,
                "w2t2": w2t,
                "wct2": wct,
                "i128": i128,
                "i64": i64,
            }
        )

    res = run_bass_kernel_spmd(nc, in_maps, core_ids=list(range(NCORES)))

    # un-permute: node = 64*sbg + 32*half + i -> outT[10*half + c, 32*sbg + i]
    out = np.empty((Npad, C), np.float32)
    n = np.arange(NP)
    col = 32 * (n // 64) + (n % 32)
    row0 = C * ((n % 64) // 32)
    for cid in range(NCORES):
        oT = res.results[cid]["outT"]
        for cls in range(C):
            out[cid * NP + n, cls] = oT[row0 + cls, col]
    return out[:N]
